# revision 9
# baseline (speedup 1.0000x reference)
"""AttentionConv (7x7 per-channel window softmax attention) on 8 Trainium2 cores.

Sharding: core = (batch b, channel-group cg).  8 cores = 4 batches x 2 channel
halves.  cg=0 cores own channels 0..31 (rel_h type), cg=1 own 32..63 (rel_w
type) -- so the rel embedding type is uniform per core and no partition split
is ever needed.

On-chip layout per core: 128 SBUF partitions = 32 channels x 4 row-quarters
(14 output rows each).  Free dim = spatial pixels of that quarter.  k/v are
computed as padded "slabs" of 20 rows x 62 cols; window reads are strided APs
into the slab, so no im2col materialization is needed.

softmax uses exp without max subtraction (scores are O(1) for these inputs;
out-of-bounds window positions contribute exp(0)=1 exactly as the zero-padded
reference does).
"""

import functools
import sys
from contextlib import ExitStack

import numpy as np

sys.path.insert(0, "/opt/trn_rl_repo")

import concourse.bass as bass
import concourse.bacc as bacc
import concourse.mybir as mybir
import concourse.tile as tile
from concourse.bass_utils import run_bass_kernel_spmd

F32 = mybir.dt.float32
BF16 = mybir.dt.bfloat16
DT = BF16

WP = 62            # padded width (56 + 2*3)
SLABR = 20         # rows per quarter slab (14 + 2*3)
SLABN = SLABR * WP # 1240
CW = 392           # pixels per chunk (7 rows x 56)
EXP = mybir.ActivationFunctionType.Exp


def _mkap(t, off, dims):
    """Manual strided AP into tile t at element offset off with free dims
    [[step, count], ...] (partition dim copied from the tile)."""
    b = t[:]
    pd = list(b.ap[0])
    return bass.AP(b.tensor, b.offset + off, [pd] + [list(d) for d in dims])


def _tree_sum49(nc, e, scr, out_f32, pool_levels=0):
    """Sum the 49 CW-wide slices of e into out_f32 via contiguous-halves
    pairwise adds (bf16 2x mode).  The first pool_levels levels run on
    GPSIMD so they overlap with concurrent DVE work."""
    ev = e[:].rearrange("p (j x) -> p j x", x=CW)
    sv = scr[:].rearrange("p (j x) -> p j x", x=CW)
    eng0 = nc.gpsimd if pool_levels >= 1 else nc.vector
    eng0.tensor_add(sv[:, 0:24], ev[:, 0:24], ev[:, 24:48])
    m = 24
    lvl = 1
    while m > 3:
        h = m // 2
        eng = nc.gpsimd if pool_levels > lvl else nc.vector
        eng.tensor_add(sv[:, 0:h], sv[:, 0:h], sv[:, h:m])
        m = h
        lvl += 1
    nc.vector.tensor_add(sv[:, 0:1], sv[:, 0:1], sv[:, 1:2])
    nc.vector.tensor_add(sv[:, 0:1], sv[:, 0:1], sv[:, 2:3])
    nc.vector.tensor_add(out_f32[:], scr[:, 0:CW], e[:, 48 * CW:49 * CW])


def _body(nc, tc, ctx, x_d, w_d, rel_d, out_d):
    pool_c = ctx.enter_context(tc.tile_pool(name="const", bufs=1))
    pool_slab = ctx.enter_context(tc.tile_pool(name="slab", bufs=1))
    pool_ps = ctx.enter_context(tc.tile_pool(name="psum", bufs=3, space="PSUM"))
    pool_e = ctx.enter_context(tc.tile_pool(name="e", bufs=2))
    pool_s = ctx.enter_context(tc.tile_pool(name="scr", bufs=2))
    pool_sm = ctx.enter_context(tc.tile_pool(name="small", bufs=2))

    # ---- load inputs ----
    x_sb = pool_e.tile([64, 62 * 62], F32, tag="e")  # shares the e slots
    nc.sync.dma_start(x_sb[:], x_d.ap())
    w_sb = {}
    for t in "qkv":
        w_sb[t] = pool_c.tile([64, 32], F32, tag=f"w{t}", name=f"w{t}_sb")
        nc.sync.dma_start(w_sb[t][:], w_d[t].ap())
    relv = pool_c.tile([128, 7], F32, tag="rel")
    nc.sync.dma_start(relv[:], rel_d.ap())

    # ---- q/k/v 1x1 convs on PE ----
    k_sb = pool_slab.tile([128, SLABN], DT, tag="k")
    v_sb = pool_slab.tile([128, SLABN], DT, tag="v")
    for t, sb in (("k", k_sb), ("v", v_sb)):
        for chunk in range(5):  # 5 chunks of 4 slab rows (248 px)
            ps = pool_ps.tile([128, 248], F32, tag="ps248")
            for qd in range(4):
                base = (qd * 14) * WP + chunk * 248
                nc.tensor.matmul(
                    ps[32 * qd:32 * qd + 32, :], w_sb[t][:],
                    x_sb[:, base:base + 248],
                    start=True, stop=True, tile_position=(0, 32 * qd))
            nc.scalar.copy(sb[:, chunk * 248:(chunk + 1) * 248], ps[:])

    q_sb = pool_slab.tile([128, 784], DT, tag="q")
    x3 = x_sb[:].rearrange("p (r w) -> p r w", w=WP)
    for chunk in range(2):  # central pixels only, 7 rows each
        ps = pool_ps.tile([128, CW], F32, tag="ps392")
        for qd in range(4):
            r0 = qd * 14 + 3 + chunk * 7
            nc.tensor.matmul(
                ps[32 * qd:32 * qd + 32, :], w_sb["q"][:],
                x3[:, r0:r0 + 7, 3:59],
                start=True, stop=True, tile_position=(0, 32 * qd))
        nc.scalar.copy(q_sb[:, chunk * CW:(chunk + 1) * CW], ps[:])

    # ---- odd-shifted k slab (keeps DVE 2x alignment for odd kw) ----
    k_od = pool_slab.tile([128, SLABN], DT, tag="ko")
    nc.gpsimd.tensor_copy(k_od[:, 0:SLABN - 1], k_sb[:, 1:SLABN])

    # ---- v + rel slabs, one per window index t, plus odd-shifted copies ----
    vc, vco = [], []
    for t in range(7):
        a = pool_slab.tile([128, SLABN], DT, tag=f"vc{t}", name=f"vc{t}")
        nc.gpsimd.tensor_scalar_add(a[:], v_sb[:], relv[:, t:t + 1])
        b = pool_slab.tile([128, SLABN], DT, tag=f"vo{t}", name=f"vo{t}")
        nc.gpsimd.tensor_copy(b[:, 0:SLABN - 1], a[:, 1:SLABN])
        vc.append(a)
        vco.append(b)

    out_ap = out_d.ap()
    for ch in range(2):  # two 7-row chunks per quarter
        # q chunk broadcast over the kh axis via a step-0 AP
        qr4 = (q_sb[:, ch * CW:(ch + 1) * CW]
               .rearrange("p (r w) -> p r w", w=56)
               .unsqueeze(1).to_broadcast([128, 7, 7, 56]))

        e = pool_e.tile([128, 49 * CW], DT, tag="e")
        # e layout: [kh, g, x] with g = 0..3 <=> kw 0,2,4,6 ; g = 4..6 <=> kw 1,3,5
        e5 = e[:].rearrange("p (kh g r w) -> p kh g r w", kh=7, g=7, w=56)

        # scores: s = q * k(window), batched per kw over all kh
        for kw in range(7):
            g = kw // 2 if kw % 2 == 0 else 4 + kw // 2
            src, b = (k_sb, kw) if kw % 2 == 0 else (k_od, kw - 1)
            in1 = _mkap(src, (ch * 7) * WP + b, [[WP, 7], [WP, 7], [1, 56]])
            nc.vector.tensor_mul(e5[:, :, g], qr4[:], in1)

        # exp in place on ACT
        nc.scalar.activation(e[:], e[:], EXP)

        # softmax denominator
        scr = pool_s.tile([128, 24 * CW], DT, tag="scr")
        den = pool_sm.tile([128, CW], F32, tag="den")
        _tree_sum49(nc, e, scr, den, pool_levels=0)

        # e *= (v + rel)(window), in place, batched per (kh, parity)
        for kh in range(7):
            a0 = (ch * 7 + kh) * WP
            nc.vector.tensor_mul(
                e5[:, kh, 0:4], e5[:, kh, 0:4],
                _mkap(vc[kh], a0, [[2, 4], [WP, 7], [1, 56]]))
            nc.vector.tensor_mul(
                e5[:, kh, 4:7], e5[:, kh, 4:7],
                _mkap(vco[kh], a0, [[2, 3], [WP, 7], [1, 56]]))

        # numerator, then out = num / den
        num = pool_sm.tile([128, CW], F32, tag="num")
        _tree_sum49(nc, e, scr, num)
        rde = pool_sm.tile([128, CW], F32, tag="rde")
        nc.vector.reciprocal_approx_fast(rde[:], den[:])
        o = pool_sm.tile([128, CW], F32, tag="o")
        nc.vector.tensor_mul(o[:], num[:], rde[:])
        nc.sync.dma_start(out_ap[:, ch * CW:(ch + 1) * CW], o[:])


@functools.lru_cache(maxsize=1)
def _build():
    nc = bacc.Bacc("TRN2", target_bir_lowering=False, debug=False,
                   enable_asserts=False)
    x_d = nc.dram_tensor("x_pad", [64, 62 * 62], F32, kind="ExternalInput")
    w_d = {t: nc.dram_tensor(f"w{t}t", [64, 32], F32, kind="ExternalInput")
           for t in "qkv"}
    rel_d = nc.dram_tensor("relvec", [128, 7], F32, kind="ExternalInput")
    out_d = nc.dram_tensor("out", [128, 784], F32, kind="ExternalOutput")
    with tile.TileContext(nc) as tc, ExitStack() as ctx:
        _body(nc, tc, ctx, x_d, w_d, rel_d, out_d)
    nc.compile()
    return nc


def _in_maps(x, Wq, Wk, Wv, rel_h, rel_w):
    x = np.asarray(x, np.float32)
    xp = np.zeros((4, 64, 62, 62), np.float32)
    xp[:, :, 3:59, 3:59] = x
    # cg=1 cores apply rel_w, which indexes the window by kw; the kernel's
    # slab index is kh, so feed those cores a spatially transposed image
    # (the window attention itself is transpose-symmetric).
    xpt = np.ascontiguousarray(xp.transpose(0, 1, 3, 2))
    rh = np.asarray(rel_h, np.float32).reshape(32, 7)
    rw = np.asarray(rel_w, np.float32).reshape(32, 7)
    wts = {n: np.asarray(w, np.float32).T.copy()
           for n, w in (("q", Wq), ("k", Wk), ("v", Wv))}
    maps = []
    for core in range(8):
        b, cg = core // 2, core % 2
        rel = rh if cg == 0 else rw
        xi = xp if cg == 0 else xpt
        maps.append({
            "x_pad": np.ascontiguousarray(xi[b].reshape(64, 62 * 62)),
            "wqt": np.ascontiguousarray(wts["q"][:, cg * 32:(cg + 1) * 32]),
            "wkt": np.ascontiguousarray(wts["k"][:, cg * 32:(cg + 1) * 32]),
            "wvt": np.ascontiguousarray(wts["v"][:, cg * 32:(cg + 1) * 32]),
            "relvec": np.ascontiguousarray(np.tile(rel, (4, 1))),
        })
    return maps


def _assemble(results):
    out = np.empty((4, 64, 56, 56), np.float32)
    for core in range(8):
        b, cg = core // 2, core % 2
        r = results[core]["out"].reshape(4, 32, 14, 56)  # [quarter, ch, r, w]
        img = r.transpose(1, 0, 2, 3).reshape(32, 56, 56)
        if cg == 1:
            img = img.transpose(0, 2, 1)  # undo the spatial transpose
        out[b, cg * 32:(cg + 1) * 32] = img
    return out


def kernel(x, Wq, Wk, Wv, rel_h, rel_w):
    nc = _build()
    maps = _in_maps(x, Wq, Wk, Wv, rel_h, rel_w)
    res = run_bass_kernel_spmd(nc, maps, core_ids=list(range(8)))
    return _assemble(res.results)


def kernel_profiled(x, Wq, Wk, Wv, rel_h, rel_w):
    """Same as kernel() but with NTFF tracing; returns (out, exec_time_ns)."""
    nc = _build()
    maps = _in_maps(x, Wq, Wk, Wv, rel_h, rel_w)
    res = run_bass_kernel_spmd(nc, maps, core_ids=list(range(8)), trace=True)
    return _assemble(res.results), res.exec_time_ns


# revision 10
# speedup vs baseline: 1.0665x; 1.0665x over previous
"""AttentionConv (7x7 per-channel window softmax attention) on 8 Trainium2 cores.

Sharding: core = (batch b, channel-group cg).  8 cores = 4 batches x 2 channel
halves.  cg=0 cores own channels 0..31 (rel_h type), cg=1 own 32..63 (rel_w
type) -- so the rel embedding type is uniform per core and no partition split
is ever needed.

On-chip layout per core: 128 SBUF partitions = 32 channels x 4 row-quarters
(14 output rows each).  Free dim = spatial pixels of that quarter.  k/v are
computed as padded "slabs" of 20 rows x 62 cols; window reads are strided APs
into the slab, so no im2col materialization is needed.

softmax uses exp without max subtraction (scores are O(1) for these inputs;
out-of-bounds window positions contribute exp(0)=1 exactly as the zero-padded
reference does).
"""

import functools
import sys
from contextlib import ExitStack

import numpy as np

sys.path.insert(0, "/opt/trn_rl_repo")

import concourse.bass as bass
import concourse.bacc as bacc
import concourse.mybir as mybir
import concourse.tile as tile
from concourse.bass_utils import run_bass_kernel_spmd

F32 = mybir.dt.float32
BF16 = mybir.dt.bfloat16
DT = BF16

WP = 62            # padded width (56 + 2*3)
SLABR = 20         # rows per quarter slab (14 + 2*3)
SLABN = SLABR * WP # 1240
CW = 392           # pixels per chunk (7 rows x 56)
EXP = mybir.ActivationFunctionType.Exp


def _mkap(t, off, dims):
    """Manual strided AP into tile t at element offset off with free dims
    [[step, count], ...] (partition dim copied from the tile)."""
    b = t[:]
    pd = list(b.ap[0])
    return bass.AP(b.tensor, b.offset + off, [pd] + [list(d) for d in dims])


def _tree_sum49(nc, e, scr, out_f32, pool_levels=0):
    """Sum the 49 CW-wide slices of e into out_f32 via contiguous-halves
    pairwise adds (bf16 2x mode).  The first pool_levels levels run on
    GPSIMD so they overlap with concurrent DVE work."""
    ev = e[:].rearrange("p (j x) -> p j x", x=CW)
    sv = scr[:].rearrange("p (j x) -> p j x", x=CW)
    eng0 = nc.gpsimd if pool_levels >= 1 else nc.vector
    eng0.tensor_add(sv[:, 0:24], ev[:, 0:24], ev[:, 24:48])
    m = 24
    lvl = 1
    while m > 3:
        h = m // 2
        eng = nc.gpsimd if pool_levels > lvl else nc.vector
        eng.tensor_add(sv[:, 0:h], sv[:, 0:h], sv[:, h:m])
        m = h
        lvl += 1
    nc.vector.tensor_add(sv[:, 0:1], sv[:, 0:1], sv[:, 1:2])
    nc.vector.tensor_add(sv[:, 0:1], sv[:, 0:1], sv[:, 2:3])
    nc.vector.tensor_add(out_f32[:], scr[:, 0:CW], e[:, 48 * CW:49 * CW])


def _body(nc, tc, ctx, x_d, w_d, rel_d, out_d):
    pool_c = ctx.enter_context(tc.tile_pool(name="const", bufs=1))
    pool_slab = ctx.enter_context(tc.tile_pool(name="slab", bufs=1))
    pool_ps = ctx.enter_context(tc.tile_pool(name="psum", bufs=3, space="PSUM"))
    pool_e = ctx.enter_context(tc.tile_pool(name="e", bufs=2))
    pool_s = ctx.enter_context(tc.tile_pool(name="scr", bufs=2))
    pool_sm = ctx.enter_context(tc.tile_pool(name="small", bufs=2))

    # ---- load inputs ----
    x_sb = pool_e.tile([64, 62 * 62], F32, tag="e")  # shares the e slots
    nc.sync.dma_start(x_sb[:], x_d.ap())
    w_sb = {}
    for t in "qkv":
        w_sb[t] = pool_c.tile([64, 32], F32, tag=f"w{t}", name=f"w{t}_sb")
        nc.sync.dma_start(w_sb[t][:], w_d[t].ap())
    relv = pool_c.tile([128, 7], F32, tag="rel")
    nc.sync.dma_start(relv[:], rel_d.ap())

    # ---- q/k/v 1x1 convs on PE ----
    k_sb = pool_slab.tile([128, SLABN], DT, tag="k")
    v_sb = pool_slab.tile([128, SLABN], DT, tag="v")
    x3 = x_sb[:].rearrange("p (r w) -> p r w", w=WP)

    def conv_slab(t, sb):
        for chunk in range(5):  # 5 chunks of 4 slab rows (248 px)
            ps = pool_ps.tile([128, 248], F32, tag="ps248", name=f"ps_{t}{chunk}")
            for qd in range(4):
                base = (qd * 14) * WP + chunk * 248
                nc.tensor.matmul(
                    ps[32 * qd:32 * qd + 32, :], w_sb[t][:],
                    x_sb[:, base:base + 248],
                    start=True, stop=True, tile_position=(0, 32 * qd))
            nc.scalar.copy(sb[:, chunk * 248:(chunk + 1) * 248], ps[:])

    conv_slab("k", k_sb)
    # odd-shifted k slab (keeps DVE 2x alignment for odd kw)
    k_od = pool_slab.tile([128, SLABN], DT, tag="ko")
    nc.gpsimd.tensor_copy(k_od[:, 0:SLABN - 1], k_sb[:, 1:SLABN])

    q_sb = pool_slab.tile([128, 784], DT, tag="q")
    for chunk in range(2):  # central pixels only, 7 rows each
        ps = pool_ps.tile([128, CW], F32, tag="ps392", name=f"ps_q{chunk}")
        for qd in range(4):
            r0 = qd * 14 + 3 + chunk * 7
            nc.tensor.matmul(
                ps[32 * qd:32 * qd + 32, :], w_sb["q"][:],
                x3[:, r0:r0 + 7, 3:59],
                start=True, stop=True, tile_position=(0, 32 * qd))
        nc.scalar.copy(q_sb[:, chunk * CW:(chunk + 1) * CW], ps[:])

    # v side is only needed by the w-mul, well after the first scores/exp
    conv_slab("v", v_sb)
    vc, vco = [], []
    for t in range(7):
        a = pool_slab.tile([128, SLABN], DT, tag=f"vc{t}", name=f"vc{t}")
        nc.gpsimd.tensor_scalar_add(a[:], v_sb[:], relv[:, t:t + 1])
        b = pool_slab.tile([128, SLABN], DT, tag=f"vo{t}", name=f"vo{t}")
        nc.gpsimd.tensor_copy(b[:, 0:SLABN - 1], a[:, 1:SLABN])
        vc.append(a)
        vco.append(b)

    out_ap = out_d.ap()
    for ch in range(2):  # two 7-row chunks per quarter
        # q chunk broadcast over the kh axis via a step-0 AP
        qr4 = (q_sb[:, ch * CW:(ch + 1) * CW]
               .rearrange("p (r w) -> p r w", w=56)
               .unsqueeze(1).to_broadcast([128, 7, 7, 56]))

        e = pool_e.tile([128, 49 * CW], DT, tag="e")
        # e layout: [kh, g, x] with g = 0..3 <=> kw 0,2,4,6 ; g = 4..6 <=> kw 1,3,5
        e5 = e[:].rearrange("p (kh g r w) -> p kh g r w", kh=7, g=7, w=56)

        # scores: s = q * k(window), batched per kw over all kh
        for kw in range(7):
            g = kw // 2 if kw % 2 == 0 else 4 + kw // 2
            src, b = (k_sb, kw) if kw % 2 == 0 else (k_od, kw - 1)
            in1 = _mkap(src, (ch * 7) * WP + b, [[WP, 7], [WP, 7], [1, 56]])
            nc.vector.tensor_mul(e5[:, :, g], qr4[:], in1)

        # exp in place on ACT
        nc.scalar.activation(e[:], e[:], EXP)

        # softmax denominator
        scr = pool_s.tile([128, 24 * CW], DT, tag="scr")
        den = pool_sm.tile([128, CW], F32, tag="den")
        _tree_sum49(nc, e, scr, den, pool_levels=0)

        # e *= (v + rel)(window), in place, batched per (kh, parity)
        for kh in range(7):
            a0 = (ch * 7 + kh) * WP
            nc.vector.tensor_mul(
                e5[:, kh, 0:4], e5[:, kh, 0:4],
                _mkap(vc[kh], a0, [[2, 4], [WP, 7], [1, 56]]))
            nc.vector.tensor_mul(
                e5[:, kh, 4:7], e5[:, kh, 4:7],
                _mkap(vco[kh], a0, [[2, 3], [WP, 7], [1, 56]]))

        # numerator, then out = num / den
        num = pool_sm.tile([128, CW], F32, tag="num")
        _tree_sum49(nc, e, scr, num)
        rde = pool_sm.tile([128, CW], F32, tag="rde")
        nc.vector.reciprocal_approx_fast(rde[:], den[:])
        o = pool_sm.tile([128, CW], F32, tag="o")
        nc.vector.tensor_mul(o[:], num[:], rde[:])
        nc.sync.dma_start(out_ap[:, ch * CW:(ch + 1) * CW], o[:])


@functools.lru_cache(maxsize=1)
def _build():
    nc = bacc.Bacc("TRN2", target_bir_lowering=False, debug=False,
                   enable_asserts=False)
    x_d = nc.dram_tensor("x_pad", [64, 62 * 62], F32, kind="ExternalInput")
    w_d = {t: nc.dram_tensor(f"w{t}t", [64, 32], F32, kind="ExternalInput")
           for t in "qkv"}
    rel_d = nc.dram_tensor("relvec", [128, 7], F32, kind="ExternalInput")
    out_d = nc.dram_tensor("out", [128, 784], F32, kind="ExternalOutput")
    with tile.TileContext(nc) as tc, ExitStack() as ctx:
        _body(nc, tc, ctx, x_d, w_d, rel_d, out_d)
    nc.compile()
    return nc


def _in_maps(x, Wq, Wk, Wv, rel_h, rel_w):
    x = np.asarray(x, np.float32)
    xp = np.zeros((4, 64, 62, 62), np.float32)
    xp[:, :, 3:59, 3:59] = x
    # cg=1 cores apply rel_w, which indexes the window by kw; the kernel's
    # slab index is kh, so feed those cores a spatially transposed image
    # (the window attention itself is transpose-symmetric).
    xpt = np.ascontiguousarray(xp.transpose(0, 1, 3, 2))
    rh = np.asarray(rel_h, np.float32).reshape(32, 7)
    rw = np.asarray(rel_w, np.float32).reshape(32, 7)
    wts = {n: np.asarray(w, np.float32).T.copy()
           for n, w in (("q", Wq), ("k", Wk), ("v", Wv))}
    maps = []
    for core in range(8):
        b, cg = core // 2, core % 2
        rel = rh if cg == 0 else rw
        xi = xp if cg == 0 else xpt
        maps.append({
            "x_pad": np.ascontiguousarray(xi[b].reshape(64, 62 * 62)),
            "wqt": np.ascontiguousarray(wts["q"][:, cg * 32:(cg + 1) * 32]),
            "wkt": np.ascontiguousarray(wts["k"][:, cg * 32:(cg + 1) * 32]),
            "wvt": np.ascontiguousarray(wts["v"][:, cg * 32:(cg + 1) * 32]),
            "relvec": np.ascontiguousarray(np.tile(rel, (4, 1))),
        })
    return maps


def _assemble(results):
    out = np.empty((4, 64, 56, 56), np.float32)
    for core in range(8):
        b, cg = core // 2, core % 2
        r = results[core]["out"].reshape(4, 32, 14, 56)  # [quarter, ch, r, w]
        img = r.transpose(1, 0, 2, 3).reshape(32, 56, 56)
        if cg == 1:
            img = img.transpose(0, 2, 1)  # undo the spatial transpose
        out[b, cg * 32:(cg + 1) * 32] = img
    return out


def kernel(x, Wq, Wk, Wv, rel_h, rel_w):
    nc = _build()
    maps = _in_maps(x, Wq, Wk, Wv, rel_h, rel_w)
    res = run_bass_kernel_spmd(nc, maps, core_ids=list(range(8)))
    return _assemble(res.results)


def kernel_profiled(x, Wq, Wk, Wv, rel_h, rel_w):
    """Same as kernel() but with NTFF tracing; returns (out, exec_time_ns)."""
    nc = _build()
    maps = _in_maps(x, Wq, Wk, Wv, rel_h, rel_w)
    res = run_bass_kernel_spmd(nc, maps, core_ids=list(range(8)), trace=True)
    return _assemble(res.results), res.exec_time_ns


# revision 13
# speedup vs baseline: 1.1211x; 1.0511x over previous
"""AttentionConv (7x7 per-channel window softmax attention) on 8 Trainium2 cores.

Sharding: core = (batch b, channel-group cg).  8 cores = 4 batches x 2 channel
halves.  cg=0 cores own channels 0..31 (rel_h type), cg=1 own 32..63 (rel_w
type) -- so the rel embedding type is uniform per core and no partition split
is ever needed.

On-chip layout per core: 128 SBUF partitions = 32 channels x 4 row-quarters
(14 output rows each).  Free dim = spatial pixels of that quarter.  k/v are
computed as padded "slabs" of 20 rows x 62 cols; window reads are strided APs
into the slab, so no im2col materialization is needed.

softmax uses exp without max subtraction (scores are O(1) for these inputs;
out-of-bounds window positions contribute exp(0)=1 exactly as the zero-padded
reference does).
"""

import functools
import sys
from contextlib import ExitStack

import numpy as np

sys.path.insert(0, "/opt/trn_rl_repo")

import concourse.bass as bass
import concourse.bacc as bacc
import concourse.mybir as mybir
import concourse.tile as tile
from concourse.bass_utils import run_bass_kernel_spmd

F32 = mybir.dt.float32
BF16 = mybir.dt.bfloat16
DT = BF16

WP = 62            # padded width (56 + 2*3)
SLABR = 20         # rows per quarter slab (14 + 2*3)
SLABN = SLABR * WP # 1240
CW = 392           # pixels per chunk (7 rows x 56)
EXP = mybir.ActivationFunctionType.Exp


def _mkap(t, off, dims):
    """Manual strided AP into tile t at element offset off with free dims
    [[step, count], ...] (partition dim copied from the tile)."""
    b = t[:]
    pd = list(b.ap[0])
    return bass.AP(b.tensor, b.offset + off, [pd] + [list(d) for d in dims])


def _tree_sum49(nc, e, scr, out_f32, pool_levels=0):
    """Sum the 49 CW-wide slices of e into out_f32 via contiguous-halves
    pairwise adds (bf16 2x mode).  The first pool_levels levels run on
    GPSIMD so they overlap with concurrent DVE work."""
    ev = e[:].rearrange("p (j x) -> p j x", x=CW)
    sv = scr[:].rearrange("p (j x) -> p j x", x=CW)
    # L1 in two halves so each can start as soon as its exp half lands
    nc.vector.tensor_add(sv[:, 0:12], ev[:, 0:12], ev[:, 12:24])
    nc.vector.tensor_add(sv[:, 12:24], ev[:, 24:36], ev[:, 36:48])
    m = 24
    lvl = 1
    while m > 3:
        h = m // 2
        eng = nc.gpsimd if pool_levels > lvl else nc.vector
        eng.tensor_add(sv[:, 0:h], sv[:, 0:h], sv[:, h:m])
        m = h
        lvl += 1
    nc.vector.tensor_add(sv[:, 0:1], sv[:, 0:1], sv[:, 1:2])
    nc.vector.tensor_add(sv[:, 0:1], sv[:, 0:1], sv[:, 2:3])
    nc.vector.tensor_add(out_f32[:], scr[:, 0:CW], e[:, 48 * CW:49 * CW])


def _body(nc, tc, ctx, x_d, w_d, rel_d, out_d):
    pool_c = ctx.enter_context(tc.tile_pool(name="const", bufs=1))
    pool_slab = ctx.enter_context(tc.tile_pool(name="slab", bufs=1))
    pool_ps = ctx.enter_context(tc.tile_pool(name="psum", bufs=3, space="PSUM"))
    pool_e = ctx.enter_context(tc.tile_pool(name="e", bufs=2))
    pool_s = ctx.enter_context(tc.tile_pool(name="scr", bufs=2))
    pool_sm = ctx.enter_context(tc.tile_pool(name="small", bufs=2))

    # ---- load inputs ----
    x_sb = pool_e.tile([64, 62 * 62], F32, tag="e")  # shares the e slots
    nc.sync.dma_start(x_sb[:], x_d.ap())
    w_sb = {}
    for t in "qkv":
        w_sb[t] = pool_c.tile([64, 32], F32, tag=f"w{t}", name=f"w{t}_sb")
        nc.sync.dma_start(w_sb[t][:], w_d[t].ap())
    relv = pool_c.tile([128, 7], F32, tag="rel")
    nc.sync.dma_start(relv[:], rel_d.ap())

    # ---- q/k/v 1x1 convs on PE ----
    k_sb = pool_slab.tile([128, SLABN], DT, tag="k")
    v_sb = pool_slab.tile([128, SLABN], DT, tag="v")
    x3 = x_sb[:].rearrange("p (r w) -> p r w", w=WP)

    def conv_slab(t, sb):
        for chunk in range(5):  # 5 chunks of 4 slab rows (248 px)
            ps = pool_ps.tile([128, 248], F32, tag="ps248", name=f"ps_{t}{chunk}")
            for qd in range(4):
                base = (qd * 14) * WP + chunk * 248
                nc.tensor.matmul(
                    ps[32 * qd:32 * qd + 32, :], w_sb[t][:],
                    x_sb[:, base:base + 248],
                    start=True, stop=True, tile_position=(0, 32 * qd))
            nc.scalar.copy(sb[:, chunk * 248:(chunk + 1) * 248], ps[:])

    conv_slab("k", k_sb)
    # odd-shifted k slab (keeps DVE 2x alignment for odd kw)
    k_od = pool_slab.tile([128, SLABN], DT, tag="ko")
    nc.gpsimd.tensor_copy(k_od[:, 0:SLABN - 1], k_sb[:, 1:SLABN])

    q_sb = pool_slab.tile([128, 784], DT, tag="q")
    for chunk in range(2):  # central pixels only, 7 rows each
        ps = pool_ps.tile([128, CW], F32, tag="ps392", name=f"ps_q{chunk}")
        for qd in range(4):
            r0 = qd * 14 + 3 + chunk * 7
            nc.tensor.matmul(
                ps[32 * qd:32 * qd + 32, :], w_sb["q"][:],
                x3[:, r0:r0 + 7, 3:59],
                start=True, stop=True, tile_position=(0, 32 * qd))
        nc.scalar.copy(q_sb[:, chunk * CW:(chunk + 1) * CW], ps[:])

    # v side is only needed by the w-mul, well after the first scores/exp
    conv_slab("v", v_sb)
    vc, vco = [], []
    for t in range(7):
        a = pool_slab.tile([128, SLABN], DT, tag=f"vc{t}", name=f"vc{t}")
        nc.gpsimd.tensor_scalar_add(a[:], v_sb[:], relv[:, t:t + 1])
        b = pool_slab.tile([128, SLABN], DT, tag=f"vo{t}", name=f"vo{t}")
        nc.gpsimd.tensor_copy(b[:, 0:SLABN - 1], a[:, 1:SLABN])
        vc.append(a)
        vco.append(b)

    out_ap = out_d.ap()
    for ch in range(2):  # two 7-row chunks per quarter
        # q chunk broadcast over the kh axis via a step-0 AP
        qr4 = (q_sb[:, ch * CW:(ch + 1) * CW]
               .rearrange("p (r w) -> p r w", w=56)
               .unsqueeze(1).to_broadcast([128, 7, 7, 56]))

        e = pool_e.tile([128, 49 * CW], DT, tag="e")
        # e layout: [kh, g, x] with g = 0..3 <=> kw 0,2,4,6 ; g = 4..6 <=> kw 1,3,5
        e5 = e[:].rearrange("p (kh g r w) -> p kh g r w", kh=7, g=7, w=56)

        # scores: s = q * k(window), batched per kw over all kh
        for kw in range(7):
            g = kw // 2 if kw % 2 == 0 else 4 + kw // 2
            src, b = (k_sb, kw) if kw % 2 == 0 else (k_od, kw - 1)
            in1 = _mkap(src, (ch * 7) * WP + b, [[WP, 7], [WP, 7], [1, 56]])
            nc.vector.tensor_mul(e5[:, :, g], qr4[:], in1)

        # exp in place on ACT, split so downstream reads can start earlier
        nc.scalar.activation(e[:, 0:24 * CW], e[:, 0:24 * CW], EXP)
        nc.scalar.activation(e[:, 24 * CW:49 * CW], e[:, 24 * CW:49 * CW], EXP)

        # softmax denominator
        scr = pool_s.tile([128, 24 * CW], DT, tag="scr")
        den = pool_sm.tile([128, CW], F32, tag="den")
        _tree_sum49(nc, e, scr, den, pool_levels=0)

        # e *= (v + rel)(window), in place, batched per (kh, parity)
        for kh in range(7):
            a0 = (ch * 7 + kh) * WP
            nc.vector.tensor_mul(
                e5[:, kh, 0:4], e5[:, kh, 0:4],
                _mkap(vc[kh], a0, [[2, 4], [WP, 7], [1, 56]]))
            nc.vector.tensor_mul(
                e5[:, kh, 4:7], e5[:, kh, 4:7],
                _mkap(vco[kh], a0, [[2, 3], [WP, 7], [1, 56]]))

        # numerator, then out = num / den
        num = pool_sm.tile([128, CW], F32, tag="num")
        _tree_sum49(nc, e, scr, num)
        rde = pool_sm.tile([128, CW], F32, tag="rde")
        nc.vector.reciprocal_approx_fast(rde[:], den[:])
        o = pool_sm.tile([128, CW], F32, tag="o")
        nc.vector.tensor_mul(o[:], num[:], rde[:])
        nc.sync.dma_start(out_ap[:, ch * CW:(ch + 1) * CW], o[:])


@functools.lru_cache(maxsize=1)
def _build():
    nc = bacc.Bacc("TRN2", target_bir_lowering=False, debug=False,
                   enable_asserts=False)
    x_d = nc.dram_tensor("x_pad", [64, 62 * 62], F32, kind="ExternalInput")
    w_d = {t: nc.dram_tensor(f"w{t}t", [64, 32], F32, kind="ExternalInput")
           for t in "qkv"}
    rel_d = nc.dram_tensor("relvec", [128, 7], F32, kind="ExternalInput")
    out_d = nc.dram_tensor("out", [128, 784], F32, kind="ExternalOutput")
    with tile.TileContext(nc) as tc, ExitStack() as ctx:
        _body(nc, tc, ctx, x_d, w_d, rel_d, out_d)
    nc.compile()
    return nc


def _in_maps(x, Wq, Wk, Wv, rel_h, rel_w):
    x = np.asarray(x, np.float32)
    xp = np.zeros((4, 64, 62, 62), np.float32)
    xp[:, :, 3:59, 3:59] = x
    # cg=1 cores apply rel_w, which indexes the window by kw; the kernel's
    # slab index is kh, so feed those cores a spatially transposed image
    # (the window attention itself is transpose-symmetric).
    xpt = np.ascontiguousarray(xp.transpose(0, 1, 3, 2))
    rh = np.asarray(rel_h, np.float32).reshape(32, 7)
    rw = np.asarray(rel_w, np.float32).reshape(32, 7)
    wts = {n: np.asarray(w, np.float32).T.copy()
           for n, w in (("q", Wq), ("k", Wk), ("v", Wv))}
    maps = []
    for core in range(8):
        b, cg = core // 2, core % 2
        rel = rh if cg == 0 else rw
        xi = xp if cg == 0 else xpt
        maps.append({
            "x_pad": np.ascontiguousarray(xi[b].reshape(64, 62 * 62)),
            "wqt": np.ascontiguousarray(wts["q"][:, cg * 32:(cg + 1) * 32]),
            "wkt": np.ascontiguousarray(wts["k"][:, cg * 32:(cg + 1) * 32]),
            "wvt": np.ascontiguousarray(wts["v"][:, cg * 32:(cg + 1) * 32]),
            "relvec": np.ascontiguousarray(np.tile(rel, (4, 1))),
        })
    return maps


def _assemble(results):
    out = np.empty((4, 64, 56, 56), np.float32)
    for core in range(8):
        b, cg = core // 2, core % 2
        r = results[core]["out"].reshape(4, 32, 14, 56)  # [quarter, ch, r, w]
        img = r.transpose(1, 0, 2, 3).reshape(32, 56, 56)
        if cg == 1:
            img = img.transpose(0, 2, 1)  # undo the spatial transpose
        out[b, cg * 32:(cg + 1) * 32] = img
    return out


def kernel(x, Wq, Wk, Wv, rel_h, rel_w):
    nc = _build()
    maps = _in_maps(x, Wq, Wk, Wv, rel_h, rel_w)
    res = run_bass_kernel_spmd(nc, maps, core_ids=list(range(8)))
    return _assemble(res.results)


def kernel_profiled(x, Wq, Wk, Wv, rel_h, rel_w):
    """Same as kernel() but with NTFF tracing; returns (out, exec_time_ns)."""
    nc = _build()
    maps = _in_maps(x, Wq, Wk, Wv, rel_h, rel_w)
    res = run_bass_kernel_spmd(nc, maps, core_ids=list(range(8)), trace=True)
    return _assemble(res.results), res.exec_time_ns
